# revision 12
# baseline (speedup 1.0000x reference)
"""DINO loss kernel for Trainium2 (8 NeuronCores, Bass/Tile).

Math
----
Reference computes, with q = log_softmax(student/ts) [Ns=1280, D] and
p = softmax((teacher-center)/tt) [Nt=256, D]:

    loss = sum_{i != j} ( -sum_d p[i,d] q[j,d] ) / (Nt*Ns - Nt)

The full-pair sum factorizes over d:

    sum_{i,j} ce[i,j] = -sum_d P[d] * Q[d]
      P[d] = sum_i p[i,d]                (teacher prob column sums)
      Q[d] = sum_j q[j,d] = S[d]/ts - C  (S = raw student logit column sums,
                                          C = sum_j logsumexp_j(x/ts))
    diag  = sum_i sum_d p[i,d] q_g[i,d]
          = sum_i v_i/(ts*Z_i) - C_g     (v_i = sum_d e_t[i,d]*sg[i,d])

    loss = ( -(dot(P,S)/ts - C*sum(P)) + diag ) / (Nt*Ns - Nt)

So the device only does streaming reductions (no [Nt,Ns,D] einsum):
per-row max / sum-exp stats, raw column sums, teacher-prob column sums,
and the elementwise teacher*student_global dot for the diagonal.

Sharding (8 cores)
------------------
Pure data parallel over rows, one NEFF run, no collectives:
  core c gets student_local rows [128c,128c+128)           -> sl  [128, 65536]
           student_global rows [32c,32c+32) row-split x4   -> sg  [128, 16384]
           teacher rows        [32c,32c+32) row-split x4   -> t   [128, 16384]
Row-split x4: row i of a [32, 65536] slice is spread over partitions
4i..4i+3, 16384 columns each (a plain reshape(128, 16384) on the host),
so all engines run at full 128-partition width.

Column sums go through the PE (mask-weighted ones-matmuls into PSUM,
quarters layout [4, 16384]), softmax stats come from DVE reduce_max +
ACT exp with fused accum_out.  All cross-core / cross-partition-group
merging is exact log-sum-exp math on the host in float64.
"""

import numpy as np

import concourse.bass as bass
import concourse.bacc as bacc
import concourse.tile as tile
from concourse import mybir
from concourse.bass_utils import run_bass_kernel_spmd

F32 = mybir.dt.float32
F32R = mybir.dt.float32r
AX = mybir.AxisListType

N_CORES = 8
D = 65536
N_T = 256
N_G = 256
N_L = 1024
SL_ROWS = N_L // N_CORES          # 128 student_local rows per core
SG_ROWS = N_G // N_CORES          # 32 student_global rows per core
T_ROWS = N_T // N_CORES           # 32 teacher rows per core


def _masks(P=128):
    # M=32 masks: matmul output must cover a full 32-row block so the PSUM
    # bank is fully written (rows past the 4 real ones get zeros).
    # qmask[p, m] = 1 if m == p % 4   (row-split quarter column sums)
    qmask = np.zeros((P, 32), np.float32)
    qmask[np.arange(P), np.arange(P) % 4] = 1.0
    # emask block q ([:, 32q:32q+32]) has ones only in column q: lhsT that
    # adds a plain colsum into row q of a 32-row PSUM block.
    emask = np.zeros((P, 128), np.float32)
    for q in range(4):
        emask[:, 32 * q + q] = 1.0
    return qmask, emask


def build_nc(D=D, n_sl_chunks=16, ts=0.1, tt=0.04):
    """Build the per-core Bass program. All 8 cores run this same NEFF.

    Column sums land in PSUM as [4, 512] regions. DMA cannot read PSUM, so
    four regions are packed into one [128, 512] PSUM bank at partition bases
    0/32/64/96 (matmul out-partition offset), retired by one full-width DVE
    copy to SBUF, then 4 small DMAs to DRAM.
    """
    DQ = D // 4                    # columns per quarter
    CQ = DQ // n_sl_chunks         # sl chunk columns per quarter
    reg = 512                      # matmul free size (fp32 max / one bank)
    assert CQ % reg == 0
    rpc = CQ // reg                # regions per sl chunk
    bank_n = 4 * reg               # quarter-cols retired per PSUM bank
    assert DQ % bank_n == 0
    cpt = bank_n // CQ             # sl chunks per bank
    assert cpt * CQ == bank_n
    cht = DQ // 4                  # teacher/sg activation chunk size

    nc = bacc.Bacc()
    sl = nc.dram_tensor("sl", [128, D], F32, kind="ExternalInput")
    sg = nc.dram_tensor("sg", [128, DQ], F32, kind="ExternalInput")
    t = nc.dram_tensor("t", [128, DQ], F32, kind="ExternalInput")

    qmask_np, emask_np = _masks()
    qmask_d = nc.inline_tensor(qmask_np, name="qmask_c")
    emask_d = nc.inline_tensor(emask_np, name="emask_c")

    s_sl = nc.dram_tensor("s_sl", [4, DQ], F32, kind="ExternalOutput")
    s_sg = nc.dram_tensor("s_sg", [4, DQ], F32, kind="ExternalOutput")
    p_out = nc.dram_tensor("p_out", [4, DQ], F32, kind="ExternalOutput")
    m_sl = nc.dram_tensor("m_sl", [128, n_sl_chunks], F32, kind="ExternalOutput")
    w_sl = nc.dram_tensor("w_sl", [128, n_sl_chunks], F32, kind="ExternalOutput")
    m_sg = nc.dram_tensor("m_sg", [128, 1], F32, kind="ExternalOutput")
    w_sg = nc.dram_tensor("w_sg", [128, 4], F32, kind="ExternalOutput")
    z_t = nc.dram_tensor("z_t", [128, 4], F32, kind="ExternalOutput")
    v_t = nc.dram_tensor("v_t", [128, 4], F32, kind="ExternalOutput")

    with tile.TileContext(nc) as tc:
        with (
            tc.tile_pool(name="singles", bufs=1) as singles,
            tc.tile_pool(name="big", bufs=1) as big,
            tc.tile_pool(name="chunks", bufs=2) as chunks,
            tc.tile_pool(name="stats", bufs=1) as stats,
            tc.tile_pool(name="stage", bufs=3) as stage_pool,
            tc.tile_pool(name="psum", bufs=3, space="PSUM") as psum,
        ):
            qmask = singles.tile([128, 32], F32)
            nc.sync.dma_start(out=qmask, in_=qmask_d[:, :])
            emask = singles.tile([128, 128], F32)
            nc.sync.dma_start(out=emask, in_=emask_d[:, :])

            sgr = big.tile([128, DQ], F32)
            nc.sync.dma_start(out=sgr, in_=sg[:, :])
            tr = big.tile([128, DQ], F32)
            nc.sync.dma_start(out=tr, in_=t[:, :])

            mS = stats.tile([128, n_sl_chunks], F32)
            wS = stats.tile([128, n_sl_chunks], F32)

            def retire_bank(bank, dst, bank_i):
                """PSUM bank [128,512] -> SBUF -> 4x [4,512] DMAs to DRAM."""
                st = stage_pool.tile([128, reg], F32, tag="stage")
                nc.vector.tensor_copy(out=st, in_=bank)
                for j in range(4):
                    r = 4 * bank_i + j
                    nc.sync.dma_start(
                        out=dst[:, r * reg : (r + 1) * reg],
                        in_=st[32 * j : 32 * j + 4, :],
                    )

            # ---- student_local: stream quarter-striped chunks ----
            # chunk k holds cols {q*DQ + k*CQ + [0,CQ)} for q in 0..3
            slv = sl.rearrange("p (q k c) -> p q k c", q=4, k=n_sl_chunks)
            for bank_i in range(DQ // bank_n):
                bank = psum.tile([128, reg], F32, tag="bank")
                for kk in range(cpt):
                    k = bank_i * cpt + kk
                    ch = chunks.tile([128, 4, CQ], F32, tag="chunk")
                    nc.sync.dma_start(out=ch, in_=slv[:, :, k, :])
                    nc.vector.reduce_max(mS[:, k : k + 1], ch, axis=AX.XY)
                    nb = chunks.tile([128, 1], F32, tag="nb")
                    nc.vector.tensor_scalar_mul(nb, mS[:, k : k + 1], -1.0 / ts)
                    # colsums of the raw chunk (must precede in-place exp)
                    for s in range(rpc):
                        rl = kk * rpc + s  # region-in-bank 0..3
                        for q in range(4):
                            nc.tensor.matmul(
                                bank[32 * rl : 32 * rl + 32, :],
                                emask[:, 32 * q : 32 * q + 32],
                                ch[:, q, s * reg : (s + 1) * reg],
                                start=(q == 0),
                                stop=(q == 3),
                                tile_position=(0, 32 * rl),
                            )
                    nc.scalar.activation(
                        ch, ch, mybir.ActivationFunctionType.Exp,
                        bias=nb, scale=1.0 / ts,
                        accum_out=wS[:, k : k + 1],
                    )
                retire_bank(bank, s_sl, bank_i)

            # ---- student_global: resident [128, DQ] ----
            mG = stats.tile([128, 1], F32)
            nc.vector.reduce_max(mG, sgr, axis=AX.X)
            nG = stats.tile([128, 1], F32)
            nc.vector.tensor_scalar_mul(nG, mG, -1.0 / ts)
            for bank_i in range(DQ // bank_n):
                bank = psum.tile([128, reg], F32, tag="bank")
                for j in range(4):
                    lo = (4 * bank_i + j) * reg
                    nc.tensor.matmul(
                        bank[32 * j : 32 * j + 32, :],
                        qmask, sgr[:, lo : lo + reg],
                        start=True, stop=True,
                        tile_position=(0, 32 * j),
                    )
                retire_bank(bank, s_sg, bank_i)

            # ---- teacher: resident [128, DQ] ----
            mT = stats.tile([128, 1], F32)
            nc.vector.reduce_max(mT, tr, axis=AX.X)
            # fold per-partition maxes to per-row (groups of 4) and broadcast
            tp4 = stats.tile([32, 4], F32)
            nc.sync.dma_start(out=tp4, in_=mT)
            mrow = stats.tile([32, 1], F32)
            nc.vector.reduce_max(mrow, tp4, axis=AX.X)
            mb = stats.tile([128, 1], F32)
            nc.sync.dma_start(
                out=mb,
                in_=bass.AP(tensor=mrow.tensor, offset=mrow.offset,
                            ap=[[1, 32], [0, 4]]),
            )
            nmb = stats.tile([128, 1], F32)
            nc.vector.tensor_scalar_mul(nmb, mb, -1.0 / tt)
            zT = stats.tile([128, 4], F32)
            for j in range(4):
                nc.scalar.activation(
                    tr[:, j * cht : (j + 1) * cht],
                    tr[:, j * cht : (j + 1) * cht],
                    mybir.ActivationFunctionType.Exp,
                    bias=nmb, scale=1.0 / tt,
                    accum_out=zT[:, j : j + 1],
                )
            zloc = stats.tile([128, 1], F32)
            nc.vector.reduce_sum(zloc, zT, axis=AX.X)
            tz4 = stats.tile([32, 4], F32)
            nc.sync.dma_start(out=tz4, in_=zloc)
            z32 = stats.tile([32, 1], F32)
            nc.vector.reduce_sum(z32, tz4, axis=AX.X)
            rz32 = stats.tile([32, 1], F32)
            nc.vector.reciprocal(rz32, z32)
            rzb = stats.tile([128, 1], F32)
            nc.sync.dma_start(
                out=rzb,
                in_=bass.AP(tensor=rz32.tensor, offset=rz32.offset,
                            ap=[[1, 32], [0, 4]]),
            )
            wq = stats.tile([128, 32], F32)
            nc.vector.tensor_scalar_mul(wq, qmask, rzb)

            # P = 1/Z-weighted column sums of exp'd teacher
            for bank_i in range(DQ // bank_n):
                bank = psum.tile([128, reg], F32, tag="bank")
                for j in range(4):
                    lo = (4 * bank_i + j) * reg
                    nc.tensor.matmul(
                        bank[32 * j : 32 * j + 32, :],
                        wq, tr[:, lo : lo + reg],
                        start=True, stop=True,
                        tile_position=(0, 32 * j),
                    )
                retire_bank(bank, p_out, bank_i)

            # v_hat = per-partition dot(exp'd teacher, raw student_global).
            # (tensor_tensor_reduce dies at runtime on this stack; use a
            # gpsimd multiply + DVE reduce instead — gpsimd is idle anyway.)
            vT = stats.tile([128, 4], F32)
            for j in range(4):
                scr = chunks.tile([128, cht], F32, tag="chunk")
                nc.gpsimd.tensor_mul(
                    scr,
                    tr[:, j * cht : (j + 1) * cht],
                    sgr[:, j * cht : (j + 1) * cht],
                )
                nc.vector.reduce_sum(vT[:, j : j + 1], scr, axis=AX.X)

            # student_global exp (in-place; after v_hat reads raw sgr)
            wG = stats.tile([128, 4], F32)
            for j in range(4):
                nc.scalar.activation(
                    sgr[:, j * cht : (j + 1) * cht],
                    sgr[:, j * cht : (j + 1) * cht],
                    mybir.ActivationFunctionType.Exp,
                    bias=nG, scale=1.0 / ts,
                    accum_out=wG[:, j : j + 1],
                )

            nc.sync.dma_start(out=m_sl[:, :], in_=mS)
            nc.sync.dma_start(out=w_sl[:, :], in_=wS)
            nc.sync.dma_start(out=m_sg[:, :], in_=mG)
            nc.sync.dma_start(out=w_sg[:, :], in_=wG)
            nc.sync.dma_start(out=z_t[:, :], in_=zT)
            nc.sync.dma_start(out=v_t[:, :], in_=vT)

    nc.compile()
    return nc


_NC_CACHE = {}


def _get_nc(ts, tt):
    key = (round(ts, 9), round(tt, 9))
    if key not in _NC_CACHE:
        _NC_CACHE[key] = build_nc(ts=ts, tt=tt)
    return _NC_CACHE[key]


def _merge(results, ts, tt, n_sl_chunks=16):
    """Host-side exact merge of per-core device outputs (float64)."""
    S = np.zeros(D, np.float64)
    P = np.zeros(D, np.float64)
    C = 0.0       # sum of all student row logsumexps
    C_g = 0.0     # global-student-row portion
    diag1 = 0.0   # sum_i v_i / (ts * Z_i)
    for r in results:
        S += r["s_sl"].astype(np.float64).reshape(-1)
        S += r["s_sg"].astype(np.float64).reshape(-1)
        P += r["p_out"].astype(np.float64).reshape(-1)
        # student_local rows: per-chunk (max, sumexp) -> row lse
        m = r["m_sl"].astype(np.float64) / ts          # [128, nch]
        w = r["w_sl"].astype(np.float64)               # [128, nch]
        mx = m.max(axis=1, keepdims=True)
        lse = mx[:, 0] + np.log((w * np.exp(m - mx)).sum(axis=1))
        C += lse.sum()
        # student_global rows: per-partition lse -> merge groups of 4
        mg = r["m_sg"].astype(np.float64)[:, 0] / ts   # [128]
        wg = r["w_sg"].astype(np.float64).sum(axis=1)  # [128]
        lp = (mg + np.log(wg)).reshape(32, 4)
        mxg = lp.max(axis=1, keepdims=True)
        lse_g = mxg[:, 0] + np.log(np.exp(lp - mxg).sum(axis=1))
        C += lse_g.sum()
        C_g += lse_g.sum()
        # teacher diagonal: v_i / Z_i (common per-row exp offset cancels)
        v = r["v_t"].astype(np.float64).sum(axis=1).reshape(32, 4).sum(axis=1)
        z = r["z_t"].astype(np.float64).sum(axis=1).reshape(32, 4).sum(axis=1)
        diag1 += (v / z).sum() / ts

    cross = P @ S / ts - C * P.sum()
    diag = diag1 - C_g
    total = -cross + diag
    n_s = N_G + N_L
    n_loss_terms = N_T * n_s - min(N_T, n_s)
    return total / n_loss_terms


def kernel(out_student_global, out_student_local, out_teacher, center,
           temp_student, temp_teacher, cent_rate_m):
    out_student_global = np.asarray(out_student_global)
    out_student_local = np.asarray(out_student_local)
    out_teacher = np.asarray(out_teacher)
    center = np.asarray(center)
    ts = float(np.asarray(temp_student).reshape(-1)[0])
    tt = float(np.asarray(temp_teacher).reshape(-1)[0])

    teacher = out_teacher
    if np.any(center):
        teacher = out_teacher - center.reshape(1, -1).astype(np.float32)
    teacher = np.ascontiguousarray(teacher, dtype=np.float32)
    sg_full = np.ascontiguousarray(out_student_global, dtype=np.float32)
    sl_full = np.ascontiguousarray(out_student_local, dtype=np.float32)

    nc = _get_nc(ts, tt)
    in_maps = []
    for c in range(N_CORES):
        in_maps.append({
            "sl": sl_full[c * SL_ROWS:(c + 1) * SL_ROWS],
            "sg": sg_full[c * SG_ROWS:(c + 1) * SG_ROWS].reshape(128, D // 4),
            "t": teacher[c * T_ROWS:(c + 1) * T_ROWS].reshape(128, D // 4),
        })
    res = run_bass_kernel_spmd(nc, in_maps, core_ids=list(range(N_CORES)))
    loss = _merge(res.results, ts, tt)
    return np.float32(loss)


# revision 27
# speedup vs baseline: 73.5295x; 73.5295x over previous
"""DINO loss kernel for Trainium2 (8 NeuronCores, Bass/Tile).

Math
----
Reference computes, with q = log_softmax(student/ts) [Ns=1280, D] and
p = softmax((teacher-center)/tt) [Nt=256, D]:

    loss = sum_{i != j} ( -sum_d p[i,d] q[j,d] ) / (Nt*Ns - Nt)

The full-pair sum factorizes over d:

    sum_{i,j} ce[i,j] = -sum_d P[d] * Q[d]
      P[d] = sum_i p[i,d]                (teacher prob column sums)
      Q[d] = sum_j q[j,d] = S[d]/ts - C  (S = raw student logit column sums,
                                          C = sum_j logsumexp_j(x/ts))
    diag  = sum_i sum_d p[i,d] q_g[i,d]
          = sum_i v_i/(ts*Z_i) - C_g     (v_i = sum_d e_t[i,d]*sg[i,d])

    loss = ( -(dot(P,S)/ts - C*sum(P)) + diag ) / (Nt*Ns - Nt)

So the device only does streaming reductions (no [Nt,Ns,D] einsum):
row sum-exp stats, raw column sums, teacher-prob column sums, and the
elementwise teacher*student_global dot for the diagonal.

Sharding (8 cores)
------------------
Pure data parallel over rows, one NEFF run, no collectives:
  core c gets student_local rows [128c,128c+128)           -> sl  [128, 65536]
           student_global rows [32c,32c+32) row-split x4   -> sg  [128, 16384]
           teacher rows        [32c,32c+32) row-split x4   -> t   [128, 16384]
Row-split x4: row i of a [32, 65536] slice is spread over partitions
4i..4i+3, 16384 columns each (a plain reshape(128, 16384) on the host),
so all engines run at full 128-partition width.

Implementation notes
--------------------
* Column sums run on the PE as mask-weighted matmuls in float32r (1 cyc/row
  vs 4 for fp32; requires every writer of a matmul operand to be f32r-typed,
  so the producing DMAs/activations write through f32r-bitcast APs).
* f32r matmuls only allow output partition base 0, so each PSUM tile is
  [32, 2048] holding 4 x [32, 512] regions side by side (rows 4..31 are
  zeros from the 32-wide masks); retired by one DVE copy + one [4, 2048]
  DMA per tile.
* Teacher softmax uses an exact on-device row max (cross-partition fold via
  two tiny DMAs). Student rows skip the device max pass: the exp bias is a
  host-sampled upper bound (sample max + margin) passed as input `nbs`;
  the host computes logsumexp against that same bound. If any resulting
  stat is non-finite (pathological input distribution), kernel() falls
  back to an exact numpy evaluation.
* All cross-core / cross-partition-group merging is float64 on the host.
"""

import numpy as np

import concourse.bass as bass
import concourse.bacc as bacc
import concourse.tile as tile
from concourse import mybir
from concourse.bass_utils import run_bass_kernel_spmd

F32 = mybir.dt.float32
F32R = mybir.dt.float32r
AX = mybir.AxisListType
EXP = mybir.ActivationFunctionType.Exp

N_CORES = 8
D = 65536
N_T = 256
N_G = 256
N_L = 1024
SL_ROWS = N_L // N_CORES          # 128 student_local rows per core
SG_ROWS = N_G // N_CORES          # 32 student_global rows per core
T_ROWS = N_T // N_CORES           # 32 teacher rows per core


def _masks(P=128):
    # M=32 masks: matmul output covers a full 32-row block so the PSUM
    # region is fully written (rows past the 4 real ones get zeros).
    # qmask[p, m] = 1 if m == p % 4   (row-split quarter column sums)
    qmask = np.zeros((P, 32), np.float32)
    qmask[np.arange(P), np.arange(P) % 4] = 1.0
    # emask block q ([:, 32q:32q+32]) has ones only in column q: lhsT that
    # adds a plain colsum into row q of a 32-row PSUM region.
    emask = np.zeros((P, 128), np.float32)
    for q in range(4):
        emask[:, 32 * q + q] = 1.0
    return qmask, emask


def build_nc(D=D, n_sl_chunks=16, ts=0.1, tt=0.04):
    """Build the per-core Bass program. All 8 cores run this same NEFF."""
    DQ = D // 4                    # columns per quarter
    CQ = DQ // n_sl_chunks         # sl chunk columns per quarter
    reg = 512                      # matmul free size (one PSUM bank)
    assert CQ % reg == 0
    rpc = CQ // reg                # regions per sl chunk
    bank_n = 2 * reg               # quarter-cols per PSUM tile [32, bank_n]
    assert DQ % bank_n == 0
    cpt = bank_n // CQ             # sl chunks per psum tile
    cht = DQ // 4                  # teacher/sg activation chunk size

    nc = bacc.Bacc()
    sl = nc.dram_tensor("sl", [128, D], F32, kind="ExternalInput")
    sg = nc.dram_tensor("sg", [128, DQ], F32, kind="ExternalInput")
    t = nc.dram_tensor("t", [128, DQ], F32, kind="ExternalInput")
    nbs = nc.dram_tensor("nbs", [128, 1], F32, kind="ExternalInput")

    qmask_np, emask_np = _masks()
    qmask_d = nc.inline_tensor(qmask_np, name="qmask_c")
    emask_d = nc.inline_tensor(emask_np, name="emask_c")

    s_sl = nc.dram_tensor("s_sl", [4, DQ], F32, kind="ExternalOutput")
    s_sg = nc.dram_tensor("s_sg", [4, DQ], F32, kind="ExternalOutput")
    p_out = nc.dram_tensor("p_out", [4, DQ], F32, kind="ExternalOutput")
    w_sl = nc.dram_tensor("w_sl", [128, n_sl_chunks], F32, kind="ExternalOutput")
    w_sg = nc.dram_tensor("w_sg", [128, 4], F32, kind="ExternalOutput")
    z_t = nc.dram_tensor("z_t", [128, 4], F32, kind="ExternalOutput")
    v_t = nc.dram_tensor("v_t", [128, 4], F32, kind="ExternalOutput")

    with tile.TileContext(nc) as tc:
        with (
            tc.tile_pool(name="singles", bufs=1) as singles,
            tc.tile_pool(name="big", bufs=1) as big,
            tc.tile_pool(name="chunks", bufs=3) as chunks,
            tc.tile_pool(name="escr", bufs=1) as escr,
            tc.tile_pool(name="stats", bufs=1) as stats,
            tc.tile_pool(name="stage", bufs=2) as stage_pool,
            tc.tile_pool(name="psA", bufs=2, space="PSUM") as psA,
            tc.tile_pool(name="psB", bufs=2, space="PSUM") as psB,
        ):
            # NOTE: each engine executes its instructions in emission order,
            # so this body is laid out in expected readiness order, not by
            # logical phase: teacher chain first (it gates wq -> P), then
            # student_global, then the long student_local stream.
            qmask = singles.tile([128, 32], F32)
            nc.sync.dma_start(out=qmask.bitcast(F32R), in_=qmask_d[:, :].bitcast(F32R))
            emask = singles.tile([128, 128], F32)
            nc.sync.dma_start(out=emask.bitcast(F32R), in_=emask_d[:, :].bitcast(F32R))
            nbs_t = singles.tile([128, 1], F32)
            nc.sync.dma_start(out=nbs_t, in_=nbs[:, :])

            tr = big.tile([128, DQ], F32)
            sgr = big.tile([128, DQ], F32)
            mT4 = stats.tile([128, 4], F32)
            for j in range(4):
                nc.sync.dma_start(
                    out=tr[:, j * cht : (j + 1) * cht].bitcast(F32R),
                    in_=t[:, j * cht : (j + 1) * cht].bitcast(F32R),
                )
                nc.vector.reduce_max(
                    mT4[:, j : j + 1], tr[:, j * cht : (j + 1) * cht], axis=AX.X
                )
            for j in range(4):
                nc.sync.dma_start(
                    out=sgr[:, j * cht : (j + 1) * cht].bitcast(F32R),
                    in_=sg[:, j * cht : (j + 1) * cht].bitcast(F32R),
                )

            wS = stats.tile([128, n_sl_chunks], F32)

            def retire(stpool, bank, dst, bank_i, on_act=False):
                """PSUM [32, bank_n] -> SBUF -> one [4, bank_n] DMA."""
                st = stpool.tile([32, bank_n], F32, tag="stage")
                if on_act:
                    nc.scalar.activation(st, bank,
                                         mybir.ActivationFunctionType.Copy)
                else:
                    nc.vector.tensor_copy(out=st, in_=bank)
                nc.sync.dma_start(
                    out=dst[:, bank_i * bank_n : (bank_i + 1) * bank_n],
                    in_=st[0:4, :],
                )

            # teacher row max (exact): fold partials, broadcast per row
            mT = stats.tile([128, 1], F32)
            nc.vector.reduce_max(mT, mT4, axis=AX.X)
            tp4 = stats.tile([32, 4], F32)
            nc.sync.dma_start(out=tp4, in_=mT)
            mrow = stats.tile([32, 1], F32)
            nc.vector.reduce_max(mrow, tp4, axis=AX.X)
            mb = stats.tile([128, 1], F32)
            nc.sync.dma_start(
                out=mb,
                in_=bass.AP(tensor=mrow.tensor, offset=mrow.offset,
                            ap=[[1, 32], [0, 4]]),
            )
            nmb = stats.tile([128, 1], F32)
            nc.vector.tensor_scalar_mul(nmb, mb, -1.0 / tt)
            # teacher exp (in-place, f32r) + row partial sums
            zT = stats.tile([128, 4], F32)
            for j in range(4):
                nc.scalar.activation(
                    tr[:, j * cht : (j + 1) * cht].bitcast(F32R),
                    tr[:, j * cht : (j + 1) * cht],
                    EXP, bias=nmb, scale=1.0 / tt,
                    accum_out=zT[:, j : j + 1],
                )
            # Z fold + 1/Z-weighted mask for P
            zloc = stats.tile([128, 1], F32)
            nc.vector.reduce_sum(zloc, zT, axis=AX.X)
            tz4 = stats.tile([32, 4], F32)
            nc.sync.dma_start(out=tz4, in_=zloc)
            z32 = stats.tile([32, 1], F32)
            nc.vector.reduce_sum(z32, tz4, axis=AX.X)
            rz32 = stats.tile([32, 1], F32)
            nc.vector.reciprocal(rz32, z32)
            rzb = stats.tile([128, 1], F32)
            nc.sync.dma_start(
                out=rzb,
                in_=bass.AP(tensor=rz32.tensor, offset=rz32.offset,
                            ap=[[1, 32], [0, 4]]),
            )
            wq = stats.tile([128, 32], F32)
            nc.vector.tensor_scalar_mul(wq.bitcast(F32R), qmask, rzb)

            # student_global exp stats (scratch out; sgr stays raw; same
            # host-supplied bound as student_local). Emitted one at a time,
            # woven into the schedule below (ACT executes in emission order).
            wG = stats.tile([128, 4], F32)

            def sg_exp(j):
                sc = escr.tile([128, cht], F32, tag="escr")
                nc.scalar.activation(
                    sc, sgr[:, j * cht : (j + 1) * cht],
                    EXP, bias=nbs_t, scale=1.0 / ts,
                    accum_out=wG[:, j : j + 1],
                )

            def _abank(lhsT, srct, dst, bank_i):
                bank = psA.tile([32, bank_n], F32, tag="bankA")
                for s in range(bank_n // reg):
                    lo = bank_i * bank_n + s * reg
                    nc.tensor.matmul(
                        bank[:, s * reg : (s + 1) * reg],
                        lhsT.bitcast(F32R),
                        srct[:, lo : lo + reg].bitcast(F32R),
                        start=True, stop=True,
                    )
                retire(stage_pool, bank, dst, bank_i)

            def sg_bank(bank_i):
                _abank(qmask, sgr, s_sg, bank_i)

            def p_bank(bank_i):
                _abank(wq, tr, p_out, bank_i)

            # v_hat: in-place multiply over exp'd teacher + row-sum, both on
            # DVE (gpsimd shares the SBUF port with DVE and is 2x slower;
            # tensor_tensor_reduce dies at runtime on this stack). The P
            # banks for quarter j must be emitted before vhat(j).
            vT = stats.tile([128, 4], F32)

            def vhat(j):
                nc.vector.tensor_mul(
                    tr[:, j * cht : (j + 1) * cht].bitcast(F32R),
                    tr[:, j * cht : (j + 1) * cht],
                    sgr[:, j * cht : (j + 1) * cht],
                )
                nc.vector.reduce_sum(vT[:, j : j + 1],
                                     tr[:, j * cht : (j + 1) * cht], axis=AX.X)

            # student_local bank: chunk DMA, colsum matmuls, exp on ACT
            # (exp and matmuls both read the raw chunk; exp writes a
            # throwaway scratch so they don't serialize); retire on ACT so
            # the stream has no DVE dependency at all.
            slv = sl.rearrange("p (q k c) -> p q k c", q=4, k=n_sl_chunks)

            def sl_bank(bank_i):
                bank = psB.tile([32, bank_n], F32, tag="bankB")
                for kk in range(cpt):
                    k = bank_i * cpt + kk
                    ch = chunks.tile([128, 4, CQ], F32, tag="chunk")
                    nc.sync.dma_start(
                        out=ch.bitcast(F32R), in_=slv[:, :, k, :].bitcast(F32R)
                    )
                    for s in range(rpc):
                        rl = kk * rpc + s
                        for q in range(4):
                            nc.tensor.matmul(
                                bank[:, rl * reg : (rl + 1) * reg],
                                emask[:, 32 * q : 32 * q + 32].bitcast(F32R),
                                ch[:, q, s * reg : (s + 1) * reg].bitcast(F32R),
                                start=(q == 0),
                                stop=(q == 3),
                            )
                    sc = escr.tile([128, 4 * CQ], F32, tag="escr")
                    nc.scalar.activation(
                        sc.rearrange("p (q c) -> p q c", q=4), ch, EXP,
                        bias=nbs_t, scale=1.0 / ts,
                        accum_out=wS[:, k : k + 1],
                    )
                retire(stage_pool, bank, s_sl, bank_i, on_act=True)

            # ---- interleaved schedule (per-engine order == emission) ----
            nb = DQ // bank_n
            if nb >= 16:
                sg_exp(0)
                for i in range(nb):
                    sg_bank(i)
                for i in range(0, 4):
                    sl_bank(i)
                sg_exp(1)
                for i in range(4, 6):
                    sl_bank(i)
                for i in range(0, nb // 2):
                    p_bank(i)
                vhat(0)
                vhat(1)
                for i in range(6, 8):
                    sl_bank(i)
                sg_exp(2)
                for i in range(nb // 2, nb):
                    p_bank(i)
                vhat(2)
                vhat(3)
                for i in range(8, 12):
                    sl_bank(i)
                sg_exp(3)
                for i in range(12, nb):
                    sl_bank(i)
            else:
                for j in range(4):
                    sg_exp(j)
                for i in range(nb):
                    sg_bank(i)
                for i in range(nb):
                    p_bank(i)
                for j in range(4):
                    vhat(j)
                for i in range(nb):
                    sl_bank(i)

            nc.sync.dma_start(out=w_sl[:, :], in_=wS)
            nc.sync.dma_start(out=w_sg[:, :], in_=wG)
            nc.sync.dma_start(out=z_t[:, :], in_=zT)
            nc.sync.dma_start(out=v_t[:, :], in_=vT)

    nc.compile()
    return nc


_NC_CACHE = {}


def _get_nc(ts, tt):
    key = (round(ts, 9), round(tt, 9))
    if key not in _NC_CACHE:
        _NC_CACHE[key] = build_nc(ts=ts, tt=tt)
    return _NC_CACHE[key]


def _merge(results, ts, tt, bs_scaled):
    """Host-side exact merge of per-core device outputs (float64).

    bs_scaled = b_s/ts, the (already scaled) exp bound the device used for
    student_local rows. Returns (loss, healthy).
    """
    S = np.zeros(D, np.float64)
    P = np.zeros(D, np.float64)
    C = 0.0       # sum of all student row logsumexps
    C_g = 0.0     # global-student-row portion
    diag1 = 0.0   # sum_i v_i / (ts * Z_i)
    healthy = True
    for r in results:
        S += r["s_sl"].astype(np.float64).reshape(-1)
        S += r["s_sg"].astype(np.float64).reshape(-1)
        P += r["p_out"].astype(np.float64).reshape(-1)
        # student_local rows: common bound -> lse = b/ts + log(sum w)
        w = r["w_sl"].astype(np.float64)               # [128, nch]
        wsum = w.sum(axis=1)
        healthy &= bool(np.isfinite(w).all() and (wsum > 0).all())
        C += (bs_scaled + np.log(np.maximum(wsum, 1e-300))).sum()
        # student_global rows: common bound per-partition lse -> merge 4s
        wg = r["w_sg"].astype(np.float64).sum(axis=1)  # [128]
        healthy &= bool(np.isfinite(wg).all() and (wg > 0).all())
        lp = (bs_scaled + np.log(np.maximum(wg, 1e-300))).reshape(32, 4)
        mxg = lp.max(axis=1, keepdims=True)
        lse_g = mxg[:, 0] + np.log(np.exp(lp - mxg).sum(axis=1))
        C += lse_g.sum()
        C_g += lse_g.sum()
        # teacher diagonal: v_i / Z_i (common per-row exp offset cancels)
        v = r["v_t"].astype(np.float64).sum(axis=1).reshape(32, 4).sum(axis=1)
        z = r["z_t"].astype(np.float64).sum(axis=1).reshape(32, 4).sum(axis=1)
        healthy &= bool(np.isfinite(v).all() and np.isfinite(z).all()
                        and (z > 0).all())
        diag1 += (v / np.maximum(z, 1e-300)).sum() / ts
        healthy &= bool(np.isfinite(r["s_sl"]).all()
                        and np.isfinite(r["s_sg"]).all()
                        and np.isfinite(r["p_out"]).all())

    cross = P @ S / ts - C * P.sum()
    diag = diag1 - C_g
    total = -cross + diag
    n_s = N_G + N_L
    n_loss_terms = N_T * n_s - min(N_T, n_s)
    loss = total / n_loss_terms
    healthy &= bool(np.isfinite(loss))
    return loss, healthy


def _numpy_loss(sg_full, sl_full, teacher, ts, tt):
    """Exact host fallback (never hit for sane input distributions)."""
    x = np.concatenate([sg_full, sl_full], axis=0).astype(np.float64) / ts
    lq = x - x.max(axis=1, keepdims=True)
    lq -= np.log(np.exp(lq).sum(axis=1, keepdims=True))
    y = teacher.astype(np.float64) / tt
    e = np.exp(y - y.max(axis=1, keepdims=True))
    p = e / e.sum(axis=1, keepdims=True)
    ce = -(p @ lq.T)
    n_t, n_s = ce.shape
    idx = np.arange(n_t)
    ce[idx, idx] = 0.0
    return ce.sum() / (n_t * n_s - min(n_t, n_s))


def kernel(out_student_global, out_student_local, out_teacher, center,
           temp_student, temp_teacher, cent_rate_m):
    out_student_global = np.asarray(out_student_global)
    out_student_local = np.asarray(out_student_local)
    out_teacher = np.asarray(out_teacher)
    center = np.asarray(center)
    ts = float(np.asarray(temp_student).reshape(-1)[0])
    tt = float(np.asarray(temp_teacher).reshape(-1)[0])

    teacher = out_teacher
    if np.any(center):
        teacher = out_teacher - center.reshape(1, -1).astype(np.float32)
    teacher = np.ascontiguousarray(teacher, dtype=np.float32)
    sg_full = np.ascontiguousarray(out_student_global, dtype=np.float32)
    sl_full = np.ascontiguousarray(out_student_local, dtype=np.float32)

    # Safe exp bound for student rows: strided-sample max + margin.
    smax = max(float(sl_full.ravel()[::257].max()),
               float(sg_full.ravel()[::257].max()))
    b_s = smax + 1.0
    nbs = np.full((128, 1), -b_s / ts, np.float32)

    nc = _get_nc(ts, tt)
    in_maps = []
    for c in range(N_CORES):
        in_maps.append({
            "sl": sl_full[c * SL_ROWS:(c + 1) * SL_ROWS],
            "sg": sg_full[c * SG_ROWS:(c + 1) * SG_ROWS].reshape(128, D // 4),
            "t": teacher[c * T_ROWS:(c + 1) * T_ROWS].reshape(128, D // 4),
            "nbs": nbs,
        })
    res = run_bass_kernel_spmd(nc, in_maps, core_ids=list(range(N_CORES)))
    loss, healthy = _merge(res.results, ts, tt, b_s / ts)
    if not healthy:
        loss = _numpy_loss(sg_full, sl_full, teacher, ts, tt)
    return np.float32(loss)
